# revision 9
# baseline (speedup 1.0000x reference)
"""Trainium2 Bass kernel for NonLinearMessagePassingLayer.

Strategy (8 NeuronCores, no collectives):
  - Host shards EDGES by the receiver's node shard (8 shards of N/8 nodes)
    and sorts each shard's edges by receiver. Per 128-node tile, the edge
    list is padded to a multiple of 128 so every 128-edge chunk maps to
    exactly one 128-node tile.
  - Each core: gathers sender rows (indirect DMA); receiver rows are
    reconstructed with a one-hot matmul against the resident local node
    shard (receivers are local by construction). MLPs run in transposed
    layout (features on partitions, edges on free dim) with float32r
    matmuls. Messages scatter-add into the local node aggregate via
    one-hot bf16 matmuls (PSUM) + dynamic-offset SBUF accumulation.
    Node MLP + LayerNorm then runs on the local shard.
  - Host inverse-permutes edge outputs and concatenates node shards.

Hardcoded problem shape: nodes [50000,128], edges [600000,128], H=256.
"""

import ml_dtypes
import numpy as np

import concourse.bass as bass
import concourse.tile as tile
from concourse import bacc, mybir
from concourse.bass_utils import run_bass_kernel_spmd
from concourse.masks import make_identity

F32 = mybir.dt.float32
F32R = mybir.dt.float32r
BF16 = mybir.dt.bfloat16
I32 = mybir.dt.int32

P = 128
D = 128
H = 256
N_NODES = 50000
N_EDGES = 600000
N_CORES = 8
N_SHARD = N_NODES // N_CORES  # 6250
LN_EPS = 1e-6

Act = mybir.ActivationFunctionType
Alu = mybir.AluOpType
ET = mybir.EngineType


# --------------------------------------------------------------------------
# Device kernel builder
# --------------------------------------------------------------------------
def build_kernel(e_pad: int, n_shard_pad: int, n_nodes: int, n_cores: int, ablate=()):
    assert e_pad % 512 == 0 and n_shard_pad % 512 == 0
    n_tiles = e_pad // 512
    n_chunks = e_pad // 128
    nt_tiles = n_shard_pad // 512

    nc = bacc.Bacc("TRN2", debug=False, num_devices=n_cores)

    edgesT = nc.dram_tensor("edgesT", [P, e_pad], F32R, kind="ExternalInput")
    senders = nc.dram_tensor("senders", [P, n_chunks], I32, kind="ExternalInput")
    rlocf = nc.dram_tensor("rlocf", [P, n_chunks], F32, kind="ExternalInput")
    toff = nc.dram_tensor("toff", [1, n_chunks], I32, kind="ExternalInput")
    nodes = nc.dram_tensor("nodes", [n_nodes, D], F32, kind="ExternalInput")
    nodesT_sh = nc.dram_tensor(
        "nodesT_sh", [P, n_shard_pad], F32R, kind="ExternalInput"
    )
    recvg = nc.dram_tensor("recvg", [P, n_chunks], I32, kind="ExternalInput")

    msg_w1 = nc.dram_tensor("msg_w1", [3 * D, H], F32R, kind="ExternalInput")
    msg_b1 = nc.dram_tensor("msg_b1", [H], F32, kind="ExternalInput")
    msg_w2 = nc.dram_tensor("msg_w2", [H, D], F32R, kind="ExternalInput")
    msg_b2 = nc.dram_tensor("msg_b2", [D], F32, kind="ExternalInput")
    edge_w1 = nc.dram_tensor("edge_w1", [3 * D, H], F32R, kind="ExternalInput")
    edge_b1 = nc.dram_tensor("edge_b1", [H], F32, kind="ExternalInput")
    edge_w2 = nc.dram_tensor("edge_w2", [H, D], F32R, kind="ExternalInput")
    edge_b2 = nc.dram_tensor("edge_b2", [D], F32, kind="ExternalInput")
    node_w1 = nc.dram_tensor("node_w1", [2 * D, H], F32R, kind="ExternalInput")
    node_b1 = nc.dram_tensor("node_b1", [H], F32, kind="ExternalInput")
    node_w2 = nc.dram_tensor("node_w2", [H, D], F32R, kind="ExternalInput")
    node_b2 = nc.dram_tensor("node_b2", [D], F32, kind="ExternalInput")
    wn = nc.dram_tensor("wn", [D, D], F32R, kind="ExternalInput")
    we = nc.dram_tensor("we", [D, D], F32R, kind="ExternalInput")
    ln_scale = nc.dram_tensor("ln_scale", [D], F32, kind="ExternalInput")
    ln_bias = nc.dram_tensor("ln_bias", [D], F32, kind="ExternalInput")

    edges_out = nc.dram_tensor("edges_out", [e_pad, D], F32, kind="ExternalOutput")
    nodes_out = nc.dram_tensor(
        "nodes_out", [n_shard_pad, D], F32, kind="ExternalOutput"
    )

    with tile.TileContext(nc) as tc:
        with (
            tc.tile_pool(name="const", bufs=1) as const,
            tc.tile_pool(name="io", bufs=3) as io,
            tc.tile_pool(name="work", bufs=2) as work,
            tc.tile_pool(name="small", bufs=4) as small,
            tc.tile_pool(name="psA", bufs=2, space="PSUM") as psA,
            tc.tile_pool(name="psB", bufs=2, space="PSUM") as psB,
            tc.tile_pool(
                name="psC", bufs=(3 if "psc3" in ablate else 2), space="PSUM"
            ) as psC,
            tc.tile_pool(
                name="psD", bufs=(1 if "psc3" in ablate else 2), space="PSUM"
            ) as psD,
        ):
            # ============== constants / weights ==============
            def load_w(ap, rows, cols, name):
                t = const.tile([rows, cols], F32R, tag=name)
                nc.sync.dma_start(out=t[:], in_=ap)
                return t

            w1m = [
                load_w(msg_w1.ap()[kb * P : (kb + 1) * P, :], P, H, f"w1m{kb}")
                for kb in range(3)
            ]
            w1e = [
                load_w(edge_w1.ap()[kb * P : (kb + 1) * P, :], P, H, f"w1e{kb}")
                for kb in range(3)
            ]
            w1n = [
                load_w(node_w1.ap()[kb * P : (kb + 1) * P, :], P, H, f"w1n{kb}")
                for kb in range(2)
            ]
            w2m = [
                load_w(msg_w2.ap()[hb * P : (hb + 1) * P, :], P, D, f"w2m{hb}")
                for hb in range(2)
            ]
            w2e = [
                load_w(edge_w2.ap()[hb * P : (hb + 1) * P, :], P, D, f"w2e{hb}")
                for hb in range(2)
            ]
            w2n = [
                load_w(node_w2.ap()[hb * P : (hb + 1) * P, :], P, D, f"w2n{hb}")
                for hb in range(2)
            ]
            wn_r = load_w(wn.ap()[:, :], D, D, "wn")
            we_r = load_w(we.ap()[:, :], D, D, "we")

            def bias_cols(dram, n, name):
                t = const.tile([P, n], F32, tag=f"{name}")
                nc.sync.dma_start(
                    out=t[:], in_=dram.ap().rearrange("(hb p) -> p hb", p=P)
                )
                return t

            b1m = bias_cols(msg_b1, 2, "b1m")
            b2m = bias_cols(msg_b2, 1, "b2m")
            b1e = bias_cols(edge_b1, 2, "b1e")
            b2e = bias_cols(edge_b2, 1, "b2e")
            b1n = bias_cols(node_b1, 2, "b1n")
            b2n = bias_cols(node_b2, 1, "b2n")

            def bcast_row(dram, name):
                t = const.tile([P, D], F32, tag=name)
                src = bass.AP(tensor=dram.ap().tensor, offset=0, ap=[[0, P], [1, D]])
                nc.gpsimd.dma_start(out=t[:], in_=src)
                return t

            lnsc = bcast_row(ln_scale, "lnsc")
            lnbs = bcast_row(ln_bias, "lnbs")

            def bcast4(t):
                return bass.AP(
                    tensor=t[:].tensor,
                    offset=t[:].offset,
                    ap=[t[:].ap[0], [0, 4], t[:].ap[1]],
                )

            lnsc4, lnbs4 = bcast4(lnsc), bcast4(lnbs)

            ident = const.tile([P, P], F32, tag="ident")
            make_identity(nc, ident[:])
            ident_bf = const.tile([P, P], BF16, tag="ident_bf")
            nc.vector.tensor_copy(out=ident_bf[:], in_=ident[:])

            iota_f = const.tile([P, P], F32, tag="iota_f")
            nc.gpsimd.iota(
                iota_f[:],
                pattern=[[1, P]],
                base=0,
                channel_multiplier=0,
                allow_small_or_imprecise_dtypes=True,
            )
            iota4 = bcast4(iota_f)  # [P, 4, P], j-broadcast

            eps_sb = const.tile([P, 1], F32, tag="eps")
            nc.vector.memset(eps_sb[:], LN_EPS)

            send_sb = const.tile([P, n_chunks], I32, tag="send_sb")
            nc.sync.dma_start(out=send_sb[:], in_=senders.ap())
            rloc_sb = const.tile([P, n_chunks], F32, tag="rloc_sb")
            nc.sync.dma_start(out=rloc_sb[:], in_=rlocf.ap())
            toff_sb = const.tile([1, n_chunks], I32, tag="toff_sb")
            nc.sync.dma_start(out=toff_sb[:], in_=toff.ap())

            recv_sb = const.tile([P, n_chunks], I32, tag="recv_sb")
            nc.sync.dma_start(out=recv_sb[:], in_=recvg.ap())

            agg_sb = const.tile([P, n_shard_pad], F32, tag="agg_sb")
            nc.vector.memset(agg_sb[:], 0.0)

            # LayerNorm on [P, 4, P] rows + affine + store
            def layernorm_out(pl_sb, out_dram_slice):
                mv4 = small.tile([P, 4, 2], F32, tag="mv4")
                for j in range(4):
                    stats = small.tile([P, 6], F32, tag="stats")
                    nc.vector.bn_stats(out=stats[:], in_=pl_sb[:, j, :])
                    nc.vector.bn_aggr(out=mv4[:, j, :], in_=stats[:])
                rstd4 = small.tile([P, 4], F32, tag="rstd4")
                nc.scalar.activation(
                    out=rstd4[:],
                    in_=mv4[:, :, 1],
                    func=Act.Sqrt,
                    bias=eps_sb[:],
                    scale=1.0,
                )
                nc.vector.reciprocal(out=rstd4[:], in_=rstd4[:])
                out_sb = io.tile([P, 4, P], F32, tag="out_sb")
                for j in range(4):
                    nc.vector.tensor_scalar(
                        out=out_sb[:, j, :],
                        in0=pl_sb[:, j, :],
                        scalar1=mv4[:, j, 0:1],
                        scalar2=rstd4[:, j : j + 1],
                        op0=Alu.subtract,
                        op1=Alu.mult,
                    )
                nc.vector.tensor_mul(out=out_sb[:], in0=out_sb[:], in1=lnsc4)
                nc.vector.tensor_add(out=out_sb[:], in0=out_sb[:], in1=lnbs4)
                nc.sync.dma_start(
                    out=out_dram_slice.rearrange("(j p) k -> p j k", p=P),
                    in_=out_sb[:],
                )

            # ============== edge phase ==============
            for ti in range(n_tiles):
                c0 = ti * 4

                eT_r = io.tile([P, 512], F32R, tag="eT_r")
                nc.sync.dma_start(
                    out=eT_r[:], in_=edgesT.ap()[:, ti * 512 : (ti + 1) * 512]
                )

                xs_rows = io.tile([P, 4, D], F32, tag="xs_rows")
                xr_rows = io.tile([P, 4, D], F32, tag="xr_rows")
                if "gather" not in ablate:
                    for j in range(4):
                        nc.gpsimd.indirect_dma_start(
                            out=xs_rows[:, j, :],
                            out_offset=None,
                            in_=nodes.ap(),
                            in_offset=bass.IndirectOffsetOnAxis(
                                ap=send_sb[:, c0 + j : c0 + j + 1], axis=0
                            ),
                        )
                        nc.gpsimd.indirect_dma_start(
                            out=xr_rows[:, j, :],
                            out_offset=None,
                            in_=nodes.ap(),
                            in_offset=bass.IndirectOffsetOnAxis(
                                ap=recv_sb[:, c0 + j : c0 + j + 1], axis=0
                            ),
                        )
                else:
                    nc.sync.dma_start(
                        out=xs_rows[:], in_=nodes.ap()[:512, :].rearrange("(j p) k -> p j k", p=P)
                    )
                    nc.sync.dma_start(
                        out=xr_rows[:], in_=nodes.ap()[:512, :].rearrange("(j p) k -> p j k", p=P)
                    )

                # per-chunk node-tile offsets (PE for x_r lhsT, DVE for flush)
                _, svals = nc.values_load_multi_w_load_instructions(
                    toff_sb[0:1, c0 : c0 + 4],
                    engines={ET.PE, ET.DVE},
                    min_val=0,
                    max_val=n_shard_pad - P,
                    skip_runtime_bounds_check=True,
                )

                # one-hot [e, n] for all 4 chunks in one op
                oh4 = work.tile([P, 4, P], BF16, tag="oh4")
                rloc_bc = bass.AP(
                    tensor=rloc_sb[:].tensor,
                    offset=rloc_sb[:, c0 : c0 + 4].offset,
                    ap=[rloc_sb[:].ap[0], [1, 4], [0, P]],
                )
                nc.vector.tensor_tensor(
                    out=oh4[:], in0=iota4, in1=rloc_bc, op=Alu.is_equal
                )

                # x_r^T via PE transpose of gathered rows
                xrT_ps = psA.tile([P, 512], F32, space="PSUM", tag="inT_ps")
                for j in range(4):
                    nc.tensor.transpose(
                        out=xrT_ps[:, j * P : (j + 1) * P],
                        in_=xr_rows[:, j, :],
                        identity=ident[:],
                    )
                xrT_r = work.tile([P, 512], F32R, tag="xrT_r")
                nc.vector.tensor_copy(out=xrT_r[:], in_=xrT_ps[:])

                # x_s^T via PE transpose of gathered rows
                xsT_ps = psA.tile([P, 512], F32, space="PSUM", tag="inT_ps")
                for j in range(4):
                    nc.tensor.transpose(
                        out=xsT_ps[:, j * P : (j + 1) * P],
                        in_=xs_rows[:, j, :],
                        identity=ident[:],
                    )
                xsT_r = work.tile([P, 512], F32R, tag="xsT_r")
                nc.vector.tensor_copy(out=xsT_r[:], in_=xsT_ps[:])

                inT = [xsT_r, xrT_r, eT_r]
                if "mlponly" in ablate:
                    inT = [eT_r, eT_r, eT_r]

                def mlp_hidden(w1, b1, name):
                    hs = []
                    for hb in range(2):
                        ps_h = psB.tile([P, 512], F32, space="PSUM", tag="ps_h")
                        for kb in range(3):
                            nc.tensor.matmul(
                                out=ps_h[:],
                                lhsT=w1[kb][:, hb * P : (hb + 1) * P],
                                rhs=inT[kb][:],
                                start=(kb == 0),
                                stop=(kb == 2),
                            )
                        h_r = work.tile([P, 512], F32R, tag=f"h_{name}{hb}")
                        nc.scalar.activation(
                            out=h_r[:],
                            in_=ps_h[:],
                            func=Act.Relu,
                            bias=b1[:, hb : hb + 1],
                            scale=1.0,
                        )
                        hs.append(h_r)
                    return hs

                hm = mlp_hidden(w1m, b1m, "m")
                he = mlp_hidden(w1e, b1e, "e")

                ps_m = psC.tile([P, 512], F32, space="PSUM", tag="ps_out")
                for hb in range(2):
                    nc.tensor.matmul(
                        out=ps_m[:],
                        lhsT=w2m[hb][:],
                        rhs=hm[hb][:],
                        start=(hb == 0),
                        stop=(hb == 1),
                    )
                msgT_bf = work.tile([P, 512], BF16, tag="msgT_bf")
                nc.scalar.activation(
                    out=msgT_bf[:],
                    in_=ps_m[:],
                    func=Act.Identity,
                    bias=b2m[:, 0:1],
                    scale=1.0,
                )

                ps_e = psC.tile([P, 512], F32, space="PSUM", tag="ps_out")
                for hb in range(2):
                    nc.tensor.matmul(
                        out=ps_e[:],
                        lhsT=w2e[hb][:],
                        rhs=he[hb][:],
                        start=(hb == 0),
                        stop=False,
                    )
                nc.tensor.matmul(
                    out=ps_e[:], lhsT=we_r[:], rhs=eT_r[:], start=False, stop=True
                )
                plT_sb = work.tile([P, 512], F32, tag="plT_sb")
                nc.scalar.activation(
                    out=plT_sb[:],
                    in_=ps_e[:],
                    func=Act.Identity,
                    bias=b2e[:, 0:1],
                    scale=1.0,
                )

                msg_ps = psC.tile([P, 512], BF16, space="PSUM", tag="ps_out")
                for j in range(4):
                    nc.tensor.transpose(
                        out=msg_ps[:, j * P : (j + 1) * P],
                        in_=msgT_bf[:, j * P : (j + 1) * P],
                        identity=ident_bf[:],
                    )
                msg_bf = work.tile([P, 4, P], BF16, tag="msg_bf")
                nc.vector.tensor_copy(out=msg_bf[:], in_=msg_ps[:])

                pl_ps = psC.tile([P, 512], F32, space="PSUM", tag="ps_out")
                for j in range(4):
                    nc.tensor.transpose(
                        out=pl_ps[:, j * P : (j + 1) * P],
                        in_=plT_sb[:, j * P : (j + 1) * P],
                        identity=ident[:],
                    )
                pl_sb = work.tile([P, 4, P], F32, tag="pl_sb")
                nc.vector.tensor_copy(out=pl_sb[:], in_=pl_ps[:])

                layernorm_out(pl_sb, edges_out.ap()[ti * 512 : (ti + 1) * 512, :])

                for j in range(4):
                    sc_ps = psD.tile([P, P], F32, space="PSUM", tag="sc_ps")
                    nc.tensor.matmul(
                        out=sc_ps[:],
                        lhsT=oh4[:, j, :],
                        rhs=msg_bf[:, j, :],
                        start=True,
                        stop=True,
                    )
                    if "flush" not in ablate:
                        sl = bass.ds(svals[j], P)
                        nc.vector.tensor_add(
                            out=agg_sb[:, sl], in0=agg_sb[:, sl], in1=sc_ps[:]
                        )
                    else:
                        dump = work.tile([P, P], F32, tag="dump")
                        nc.vector.tensor_copy(out=dump[:], in_=sc_ps[:])

            # ============== node phase ==============
            for nti in range(nt_tiles):
                nT_r = io.tile([P, 512], F32R, tag="eT_r")
                nc.sync.dma_start(
                    out=nT_r[:], in_=nodesT_sh.ap()[:, nti * 512 : (nti + 1) * 512]
                )

                aggT_ps = psA.tile([P, 512], F32, space="PSUM", tag="inT_ps")
                for j in range(4):
                    t128 = nti * 4 + j
                    nc.tensor.transpose(
                        out=aggT_ps[:, j * P : (j + 1) * P],
                        in_=agg_sb[:, t128 * P : (t128 + 1) * P],
                        identity=ident[:],
                    )
                aggT_r = work.tile([P, 512], F32R, tag="xsT_r")
                nc.vector.tensor_copy(out=aggT_r[:], in_=aggT_ps[:])

                inTn = [nT_r, aggT_r]
                hn = []
                for hb in range(2):
                    ps_h = psB.tile([P, 512], F32, space="PSUM", tag="ps_h")
                    for kb in range(2):
                        nc.tensor.matmul(
                            out=ps_h[:],
                            lhsT=w1n[kb][:, hb * P : (hb + 1) * P],
                            rhs=inTn[kb][:],
                            start=(kb == 0),
                            stop=(kb == 1),
                        )
                    h_r = work.tile([P, 512], F32R, tag=f"h_m{hb}")
                    nc.scalar.activation(
                        out=h_r[:],
                        in_=ps_h[:],
                        func=Act.Relu,
                        bias=b1n[:, hb : hb + 1],
                        scale=1.0,
                    )
                    hn.append(h_r)

                ps_o = psC.tile([P, 512], F32, space="PSUM", tag="ps_out")
                for hb in range(2):
                    nc.tensor.matmul(
                        out=ps_o[:],
                        lhsT=w2n[hb][:],
                        rhs=hn[hb][:],
                        start=(hb == 0),
                        stop=False,
                    )
                nc.tensor.matmul(
                    out=ps_o[:], lhsT=wn_r[:], rhs=nT_r[:], start=False, stop=True
                )
                plT_n = work.tile([P, 512], F32, tag="plT_sb")
                nc.scalar.activation(
                    out=plT_n[:],
                    in_=ps_o[:],
                    func=Act.Identity,
                    bias=b2n[:, 0:1],
                    scale=1.0,
                )

                pl_ps = psC.tile([P, 512], F32, space="PSUM", tag="ps_out")
                for j in range(4):
                    nc.tensor.transpose(
                        out=pl_ps[:, j * P : (j + 1) * P],
                        in_=plT_n[:, j * P : (j + 1) * P],
                        identity=ident[:],
                    )
                pl_sb = work.tile([P, 4, P], F32, tag="pl_sb")
                nc.vector.tensor_copy(out=pl_sb[:], in_=pl_ps[:])

                layernorm_out(pl_sb, nodes_out.ap()[nti * 512 : (nti + 1) * 512, :])

    nc.compile()
    return nc


_KERNEL_CACHE: dict = {}


def get_kernel(e_pad, n_shard_pad, n_nodes, n_cores, ablate=()):
    key = (e_pad, n_shard_pad, n_nodes, n_cores, tuple(ablate))
    if key not in _KERNEL_CACHE:
        _KERNEL_CACHE[key] = build_kernel(e_pad, n_shard_pad, n_nodes, n_cores, ablate)
    return _KERNEL_CACHE[key]


# --------------------------------------------------------------------------
# Host-side sharding / padding
# --------------------------------------------------------------------------
def _prepare_core(receivers, core, n_shard):
    base = core * n_shard
    sel = np.nonzero((receivers >= base) & (receivers < base + n_shard))[0]
    rloc = receivers[sel] - base
    order = np.argsort(rloc, kind="stable")
    sel = sel[order]
    rloc = rloc[order]

    n_tiles_128 = (n_shard + P - 1) // P
    tile_of_edge = rloc // P
    counts = np.bincount(tile_of_edge, minlength=n_tiles_128)
    padded_counts = ((counts + P - 1) // P) * P
    total = int(padded_counts.sum())
    tile_starts = np.concatenate([[0], np.cumsum(padded_counts)[:-1]])
    if len(sel):
        within = np.concatenate([np.arange(c) for c in counts])
    else:
        within = np.array([], dtype=np.int64)
    pos = (tile_starts[tile_of_edge] + within).astype(np.int64)
    return {
        "sel": sel,
        "rloc": rloc,
        "pos": pos,
        "total": total,
        "tile_starts": tile_starts,
        "padded_counts": padded_counts,
    }


def run_sharded(nodes, edges, senders, receivers, weights, n_cores, trace=False):
    n_nodes = nodes.shape[0]
    n_edges = edges.shape[0]
    n_shard = n_nodes // n_cores
    n_shard_pad = ((n_shard + 511) // 512) * 512

    preps = [_prepare_core(receivers, c, n_shard) for c in range(n_cores)]
    e_pad = max(p["total"] for p in preps)
    e_pad = ((e_pad + 511) // 512) * 512
    n_chunks = e_pad // 128

    nc = get_kernel(e_pad, n_shard_pad, n_nodes, n_cores)

    in_maps = []
    for c in range(n_cores):
        pr = preps[c]
        sel, rloc, pos = pr["sel"], pr["rloc"], pr["pos"]

        edgesT = np.zeros((P, e_pad), dtype=np.float32)
        edgesT[:, pos] = edges[sel].T
        send_pad = np.zeros(e_pad, dtype=np.int32)
        send_pad[pos] = senders[sel]
        rloc_mod = np.full(e_pad, -1.0, dtype=np.float32)
        rloc_mod[pos] = (rloc % P).astype(np.float32)

        chunk_tile = np.full(n_chunks, (n_shard_pad // P) - 1, dtype=np.int32)
        tile_starts, padded_counts = pr["tile_starts"], pr["padded_counts"]
        for t in range(len(padded_counts)):
            s = tile_starts[t] // P
            e = s + padded_counts[t] // P
            chunk_tile[s:e] = t
        toff = (chunk_tile * P).astype(np.int32)[None, :]

        shard = nodes[c * n_shard : (c + 1) * n_shard]
        nodesT_sh = np.zeros((P, n_shard_pad), dtype=np.float32)
        nodesT_sh[:, :n_shard] = shard.T
        recv_pad = np.zeros(e_pad, dtype=np.int32)
        recv_pad[pos] = receivers[sel]

        im = {
            "edgesT": edgesT,
            "senders": np.ascontiguousarray(send_pad.reshape(n_chunks, P).T),
            "rlocf": np.ascontiguousarray(rloc_mod.reshape(n_chunks, P).T),
            "recvg": np.ascontiguousarray(recv_pad.reshape(n_chunks, P).T),
            "toff": toff,
            "nodes": nodes,
            "nodesT_sh": nodesT_sh,
        }
        im.update(weights)
        in_maps.append(im)

    run_sharded.last_in_maps = in_maps
    res = run_bass_kernel_spmd(
        nc, in_maps, core_ids=list(range(n_cores)), trace=trace
    )
    run_sharded.last_results = res

    nodes_new = np.empty((n_nodes, D), dtype=np.float32)
    edges_new = np.empty((n_edges, D), dtype=np.float32)
    for c in range(n_cores):
        r = res.results[c]
        nodes_new[c * n_shard : (c + 1) * n_shard] = r["nodes_out"][:n_shard]
        pr = preps[c]
        edges_new[pr["sel"]] = r["edges_out"][pr["pos"]]
    return nodes_new, edges_new


def kernel(
    nodes,
    edges,
    senders,
    receivers,
    msg_w1,
    msg_b1,
    msg_w2,
    msg_b2,
    node_w1,
    node_b1,
    node_w2,
    node_b2,
    edge_w1,
    edge_b1,
    edge_w2,
    edge_b2,
    Wn,
    We,
    ln_scale,
    ln_bias,
):
    weights = {
        "msg_w1": np.asarray(msg_w1, np.float32),
        "msg_b1": np.asarray(msg_b1, np.float32),
        "msg_w2": np.asarray(msg_w2, np.float32),
        "msg_b2": np.asarray(msg_b2, np.float32),
        "edge_w1": np.asarray(edge_w1, np.float32),
        "edge_b1": np.asarray(edge_b1, np.float32),
        "edge_w2": np.asarray(edge_w2, np.float32),
        "edge_b2": np.asarray(edge_b2, np.float32),
        "node_w1": np.asarray(node_w1, np.float32),
        "node_b1": np.asarray(node_b1, np.float32),
        "node_w2": np.asarray(node_w2, np.float32),
        "node_b2": np.asarray(node_b2, np.float32),
        "wn": np.asarray(Wn, np.float32),
        "we": np.asarray(We, np.float32),
        "ln_scale": np.asarray(ln_scale, np.float32),
        "ln_bias": np.asarray(ln_bias, np.float32),
    }
    return run_sharded(
        np.asarray(nodes, np.float32),
        np.asarray(edges, np.float32),
        np.asarray(senders, np.int32),
        np.asarray(receivers, np.int32),
        weights,
        N_CORES,
    )
